# revision 30
# baseline (speedup 1.0000x reference)
"""Trainium2 Bass kernel for nn_CostVolume (SpatialCorrelationSampler-style).

out[b, dy*9+dx, y, x] = sum_c feat1[b,c,y,x] * feat2_pad[b,c,y+dy,x+dx]
with feat2 zero-padded by 4 on H/W, dy/dx in [0,9), B=4, C=256, H=W=96.

Sharding (8 cores): core = (b, half) -- batch x H-half (48 rows each).

Device algorithm: 2D-tiled gram blocks. Per (y-block of 16, x-tile of 8):
  stationary = f1 tile [128c, 128=(16y x 8x)]  (128-col => FWL enabled)
  moving     = f2 window [128c, 384=(24r x 16x')], 1-2 MMs per C-half
  (row-split only at f2 group boundaries; col-split L/R tiles so compute
  starts after half a group arrives).
  psum[m=(yi,xx), n=r_rel*16+x_rel] accumulated over 2 C-halves.
  PSUM -> per-block SBUF stage [128, 12*384] bf16 (cast on copy).
  Output: per (blk, 32-partition group g) one rectangular DMA
  [[4608, 32], [384, 12], [1, 192]] at (partition 32g, col 64g): the
  union window covering all 81 displacements for those 4 yi groups
  (k = 16*(yi%4) + xx + 16*dy + dx); host strips via as_strided.
"""

import numpy as np
import ml_dtypes

import concourse.bacc as bacc
import concourse.mybir as mybir
from concourse.ap import AP
from concourse.tile import TileContext
from concourse.bass_utils import run_bass_kernel_spmd

B, C, H, W = 4, 256, 96, 96
D = 4
P = 2 * D + 1     # 9 displacements per axis
HH = H // 2       # 48 rows per core
NB = 3            # y-blocks of TY
TY, TX = 16, 8
NXT = W // TX     # 12 x-tiles
XW = TX + 8       # 16-col f2 window per tile
RL = 192          # per-32-partition-group read window (union over 4 yi)
STW = NXT * 384   # staged cols per block: 4608
GNR = (24, 16, 16)       # f2 group row counts: A=rows0-23, B=24-39, C=40-55
GR0 = (0, 24, 40)        # first f2 row of each group
HW2 = 56                 # half-width of f2 col split (L=[0:56], R=[48:104])

F32 = mybir.dt.float32
BF16 = mybir.dt.bfloat16

_CACHED = {}


def _build_nc():
    nc = bacc.Bacc()
    f1 = nc.declare_dram_parameter("f1", [128, 2, NB, NXT * 128], BF16, isOutput=False)
    f2d = {}
    for g in range(3):
        for s in "lr":
            nm = f"f2{'abc'[g]}{s}"
            f2d[(g, s)] = nc.declare_dram_parameter(
                nm, [128, 2, GNR[g] * HW2], BF16, isOutput=False
            )
    out = nc.declare_dram_parameter("o", [NB, 4, 2, 32, 6, RL], BF16, isOutput=True)

    with TileContext(nc) as tc:
        with (
            tc.tile_pool(name="w", bufs=1) as wp,
            tc.tile_pool(name="m", bufs=1) as mp,
            tc.tile_pool(name="st", bufs=3) as stp,
            tc.tile_pool(name="ps", bufs=6, space="PSUM") as psp,
            tc.tile_pool(name="wu", bufs=1, space="PSUM") as wup,
        ):
            # PE warmup: dummy matmuls on a memset tile while inputs load
            cst = wp.tile([128, 512], BF16, tag="cst", name="cst")
            nc.vector.memset(cst[:, :], 0)
            wups = wup.tile([128, 512], F32, tag="wups", name="wups")
            for _ in range(10):
                nc.tensor.matmul(
                    wups[:, 0:384], lhsT=cst[:, 0:128], rhs=cst[:, 0:384],
                    start=True, stop=True,
                )

            f1t = [None] * NB
            f2t = {}

            def load_f1(blk, ch=None):
                if f1t[blk] is None:
                    f1t[blk] = wp.tile([128, 2, NXT * 128], BF16,
                                       tag=f"f1b{blk}", name=f"f1b{blk}")
                t = f1t[blk]
                if ch is None:
                    nc.sync.dma_start(out=t[:, :, :], in_=f1[:, :, blk, :])
                else:
                    nc.sync.dma_start(out=t[:, ch, :], in_=f1[:, ch, blk, :])

            def load_f2(g, s, ch=None):
                if (g, s) not in f2t:
                    f2t[(g, s)] = mp.tile([128, 2, GNR[g] * HW2], BF16,
                                          tag=f"f2{g}{s}", name=f"f2{g}{s}")
                t = f2t[(g, s)]
                if ch is None:
                    nc.sync.dma_start(out=t[:, :, :], in_=f2d[(g, s)][:, :, :])
                else:
                    nc.sync.dma_start(out=t[:, ch, :], in_=f2d[(g, s)][:, ch, :])

            # first wave split by C-half so the first ch0 matmul starts early
            load_f1(0, 0)
            load_f2(0, "l", 0)
            load_f1(0, 1)
            load_f2(0, "l", 1)
            load_f2(0, "r", 0)
            load_f2(0, "r", 1)
            load_f1(1)
            load_f2(1, "l")
            load_f2(1, "r")
            load_f1(2)
            load_f2(2, "l")
            load_f2(2, "r")

            # per block: list of (group, row0_local_in_group, nrows, psum_col0)
            mm_plan = [
                [(0, 0, 24, 0)],
                [(0, 16, 8, 0), (1, 0, 16, 128)],
                [(1, 8, 8, 0), (2, 0, 16, 128)],
            ]

            out_engines = [nc.gpsimd, nc.sync]
            ti = 0
            oi = 0
            for blk in range(NB):
                st = stp.tile([128, STW], BF16, tag="st", name="st")
                for xt in range(NXT):
                    side = "l" if xt < 6 else "r"
                    xoff = 8 * xt if xt < 6 else 8 * xt - 48
                    ps = psp.tile([128, 512], F32, tag="ps", name="ps")
                    for g, r0, nr, col0 in mm_plan[blk]:
                        mt = f2t[(g, side)]
                        gw = GNR[g] * HW2
                        for ch in range(2):
                            wt = f1t[blk]
                            lhsT = AP(
                                tensor=wt.tensor,
                                offset=wt.offset + ch * (NXT * 128) + xt * 128,
                                ap=[[2 * NXT * 128, 128], [1, 128]],
                            )
                            rhs = AP(
                                tensor=mt.tensor,
                                offset=mt.offset + ch * gw + r0 * HW2 + xoff,
                                ap=[[2 * gw, 128], [HW2, nr], [1, XW]],
                            )
                            nc.tensor.matmul(
                                ps[:, col0 : col0 + nr * XW],
                                lhsT=lhsT,
                                rhs=rhs,
                                start=(ch == 0),
                                stop=(ch == 1),
                            )
                    dst = st[:, xt * 384 : (xt + 1) * 384]
                    nc.vector.tensor_copy(dst[:, 0:192], ps[:, 0:192])
                    nc.scalar.copy(out=dst[:, 192:384], in_=ps[:, 192:384])
                    ti += 1
                for g in range(4):
                    for h in range(2):
                        src = AP(
                            tensor=st.tensor,
                            offset=st.offset + 32 * g * STW + 64 * g
                            + h * 6 * 384,
                            ap=[[STW, 32], [384, 6], [1, RL]],
                        )
                        out_engines[oi % 2].dma_start(out=out[blk, g, h], in_=src)
                        oi += 1
    nc.finalize()
    return nc


def kernel(feat1: np.ndarray, feat2: np.ndarray) -> np.ndarray:
    feat1 = np.ascontiguousarray(np.asarray(feat1, dtype=np.float32))
    feat2 = np.ascontiguousarray(np.asarray(feat2, dtype=np.float32))

    if "nc" not in _CACHED:
        _CACHED["nc"] = _build_nc()
    nc = _CACHED["nc"]

    core_ids = list(range(8))
    in_maps = []
    for core in core_ids:
        b, half = divmod(core, 2)
        f1h = feat1[b][:, half * HH : half * HH + HH, :]  # [256, 48, 96]
        # [c, blk, yi, xt, xx] -> [cl, ch, blk, (xt yi xx)]
        f1td = (
            f1h.reshape(256, NB, TY, NXT, TX)
            .transpose(0, 1, 3, 2, 4)
            .reshape(2, 128, NB, NXT * 128)
            .transpose(1, 0, 2, 3)
        )
        f2p = np.pad(feat2[b], ((0, 0), (D, D), (D, D)))[
            :, half * HH : half * HH + HH + 8, :
        ]  # [256, 56, 104]
        m = {"f1": np.ascontiguousarray(f1td.astype(ml_dtypes.bfloat16))}
        for g in range(3):
            for s, c0 in (("l", 0), ("r", 104 - HW2)):
                sl = f2p[:, GR0[g] : GR0[g] + GNR[g], c0 : c0 + HW2]
                tg = sl.reshape(2, 128, GNR[g] * HW2).transpose(1, 0, 2)
                m[f"f2{'abc'[g]}{s}"] = np.ascontiguousarray(
                    tg.astype(ml_dtypes.bfloat16)
                )
        in_maps.append(m)

    res = run_bass_kernel_spmd(nc, in_maps, core_ids)

    out = np.empty((B, P * P, H, W), np.float32)
    for core in core_ids:
        b, half = divmod(core, 2)
        o = res.results[core]["o"]  # [3, 4, 2, 32, 6, 192] bf16
        o = np.ascontiguousarray(o).astype(np.float32)
        # partition p = 32g + q; yi = 4g + q//8; xx = q%8; xt = 6h + xtl
        # k(dy,dx) = 16*(q//8) + xx + 16*dy + dx
        o7 = o.reshape(NB, 4, 2, 4, TX, 6, RL)  # [blk, g, h, yj, xx, xtl, k]
        o6 = o7.transpose(0, 1, 3, 4, 2, 5, 6)  # [blk, g, yj, xx, h, xtl, k]
        tmp = np.empty((P * P, NB, 4, 4, NXT, TX), np.float32)
        for yj in range(4):
            for xx in range(TX):
                a = o6[:, :, yj, xx, :, :, 16 * yj + xx :]  # [3,4,2,6,>=137]
                sa = a.strides
                v = np.lib.stride_tricks.as_strided(
                    a, shape=(NB, 4, 2, 6, P, P),
                    strides=(sa[0], sa[1], sa[2], sa[3], XW * sa[4], sa[4]),
                )
                # v[blk, g, h, xtl, dy, dx]
                tmp[:, :, :, yj, :, xx] = v.transpose(4, 5, 0, 1, 2, 3).reshape(
                    P * P, NB, 4, NXT
                )
        # y = blk*16 + g*4 + yj ; x = xt*8 + xx
        core_out = tmp.reshape(P * P, HH, W)
        out[b, :, half * HH : half * HH + HH, :] = core_out
    return out
